# revision 14
# baseline (speedup 1.0000x reference)
"""Trainium2 Bass kernel for hierarchical SAGPool GCN (nn_Net_40690520162873).

Strategy: data-parallel over the 128 graphs (16 per core, 8 cores). Edges never
cross graphs, so each 512-node graph is processed fully on-chip with dense
512x512 normalized-adjacency matmuls on the PE array. The adjacency count
matrix Ct[s,d] (#edges s->d) is built on the host from src/dst (pure input
layout transformation); the three normalized matrices M0/M1/M2 are built on
device (M1/M2 depend on device-computed top-k keep masks).

Math: gcn(x) = (Mt.T @ (x @ W) + b*keep) with
  Mt[s,d] = dis[s]*Ct[s,d]*dis[d] + delta_{sd}*dis[d]^2,  dis = keep/sqrt(degE+1)
Dead rows/cols of Mt are zero, so dead-node channels stay exactly zero through
convs without extra masking. Top-k per graph via pairwise-rank: rank[i] =
#{j: s[j] > s[i]} computed with rank-2 PE matmuls + step + ones-matvec.
"""
import numpy as np
from contextlib import ExitStack

import concourse.bass as bass
import concourse.tile as tile
from concourse import bacc, mybir
from concourse.bass_utils import run_bass_kernel_spmd

B, NPER = 128, 512
F = 128
N_CORES = 8
GPC = B // N_CORES  # graphs per core
K_POOL = [256, 128, 64]
NCLS = 10
BIG = 1e30
FP = mybir.dt.float32
AF = mybir.ActivationFunctionType
ALU = mybir.AluOpType

# conv layer schedule: (weight_index, has_residual) per stage
STAGES = [
    [(0, False), (1, True), (2, True), (3, False)],
    [(4, True), (5, True), (6, True), (7, True), (8, False)],
    [(9, True), (10, True), (11, True), (12, True), (13, False)],
]


def _build_program():
    nc = bacc.Bacc("TRN2", num_devices=N_CORES)

    d_xT = nc.dram_tensor("xT", [GPC * F, NPER], FP, kind="ExternalInput").ap()
    d_ct = nc.dram_tensor("ct", [GPC * NPER, NPER], FP, kind="ExternalInput").ap()
    d_dis0 = nc.dram_tensor("dis0", [GPC, NPER], FP, kind="ExternalInput").ap()
    d_wconv = nc.dram_tensor("wconv", [F, 14 * F], FP, kind="ExternalInput").ap()
    d_bconv = nc.dram_tensor("bconv", [1, 14 * F], FP, kind="ExternalInput").ap()
    d_wpool = nc.dram_tensor("wpool", [F, 3], FP, kind="ExternalInput").ap()
    d_bpool = nc.dram_tensor("bpool", [1, 3], FP, kind="ExternalInput").ap()
    d_wlin1 = nc.dram_tensor("wlin1", [F, 2 * F], FP, kind="ExternalInput").ap()
    d_wlin2 = nc.dram_tensor("wlin2", [F, 64], FP, kind="ExternalInput").ap()
    d_wlin3 = nc.dram_tensor("wlin3", [64, NCLS], FP, kind="ExternalInput").ap()
    d_bmlp = nc.dram_tensor("bmlp", [1, 3 * F], FP, kind="ExternalInput").ap()
    d_ident = nc.dram_tensor("ident", [F, F], FP, kind="ExternalInput").ap()
    d_diag = nc.dram_tensor("diagm", [F, NPER], FP, kind="ExternalInput").ap()
    d_out = nc.dram_tensor("out", [GPC, NCLS], FP, kind="ExternalOutput").ap()

    with tile.TileContext(nc) as tc, ExitStack() as ctx:
        wp = ctx.enter_context(tc.tile_pool(name="w", bufs=1))
        ctp = ctx.enter_context(tc.tile_pool(name="ctp", bufs=2))
        mp = ctx.enter_context(tc.tile_pool(name="mp", bufs=2))
        xp = ctx.enter_context(tc.tile_pool(name="xp", bufs=2))
        hp = ctx.enter_context(tc.tile_pool(name="hp", bufs=2))
        tp = ctx.enter_context(tc.tile_pool(name="tp", bufs=2))
        sk = ctx.enter_context(tc.tile_pool(name="sk", bufs=2))
        php = ctx.enter_context(tc.tile_pool(name="php", bufs=2, space=bass.MemorySpace.PSUM))
        pap = ctx.enter_context(tc.tile_pool(name="pap", bufs=2, space=bass.MemorySpace.PSUM))
        pmp = ctx.enter_context(tc.tile_pool(name="pmp", bufs=2, space=bass.MemorySpace.PSUM))
        psp = ctx.enter_context(tc.tile_pool(name="psp", bufs=2, space=bass.MemorySpace.PSUM))

        # --- persistent weights/constants ---
        wconv = wp.tile([F, 14 * F], FP)
        nc.sync.dma_start(wconv[:], d_wconv[:])
        bconv = wp.tile([1, 14 * F], FP)
        nc.sync.dma_start(bconv[:], d_bconv[:])
        wpool = wp.tile([F, 3], FP)
        nc.sync.dma_start(wpool[:], d_wpool[:])
        bpool = wp.tile([1, 3], FP)
        nc.sync.dma_start(bpool[:], d_bpool[:])
        wlin1 = wp.tile([F, 2 * F], FP)
        nc.sync.dma_start(wlin1[:], d_wlin1[:])
        wlin2 = wp.tile([F, 64], FP)
        nc.sync.dma_start(wlin2[:], d_wlin2[:])
        wlin3 = wp.tile([64, NCLS], FP)
        nc.sync.dma_start(wlin3[:], d_wlin3[:])
        bmlp = wp.tile([1, 3 * F], FP)
        nc.sync.dma_start(bmlp[:], d_bmlp[:])
        ident = wp.tile([F, F], FP)
        nc.sync.dma_start(ident[:], d_ident[:])
        diag = wp.tile([F, NPER], FP)
        nc.sync.dma_start(diag[:], d_diag[:])
        ones_row = wp.tile([1, NPER], FP)
        nc.gpsimd.memset(ones_row[:], 1.0)
        ones_col = wp.tile([F, 1], FP)
        nc.gpsimd.memset(ones_col[:], 1.0)
        R = wp.tile([F, GPC], FP)
        R2 = wp.tile([F, GPC], FP)

        def row_to_cols(row):
            """[1,512] row -> [128,4] sbuf tile (col c = chunk c transposed)."""
            pc = psp.tile([F, 4], FP, tag="ps")
            for c in range(4):
                nc.tensor.matmul(pc[:, c:c + 1], row[0:1, c * F:(c + 1) * F],
                                 ones_row[0:1, 0:1], start=True, stop=True)
            col = sk.tile([F, 4], FP)
            nc.scalar.mul(col[:], pc[:], 1.0)
            return col

        def build_m(ct_sb, dis_row):
            """Mt chunks [128, 4*512] from dis row and count matrix."""
            dis2 = sk.tile([1, NPER], FP)
            nc.vector.tensor_mul(dis2[:], dis_row[:], dis_row[:])
            d2col = row_to_cols(dis2)
            m = mp.tile([F, 4 * NPER], FP)
            for c in range(4):
                po = pmp.tile([F, NPER], FP, tag="pm")
                nc.tensor.matmul(po[:], dis_row[0:1, c * F:(c + 1) * F],
                                 dis_row[:], start=True, stop=True)
                cto = tp.tile([F, NPER], FP)
                nc.vector.tensor_mul(cto[:], ct_sb[:, c * NPER:(c + 1) * NPER], po[:])
                nc.vector.scalar_tensor_tensor(
                    m[:, c * NPER:(c + 1) * NPER], diag[:], d2col[:, c:c + 1],
                    cto[:], op0=ALU.mult, op1=ALU.add)
            return m

        def conv(x_sb, m_sb, li, keep_row, residual):
            """x_next = [x +] relu(Mt.T @ (X W) + b*keep), feat-major [128,512]."""
            ph = php.tile([F, 4, F], FP, tag="ph")
            for c in range(4):
                nc.tensor.matmul(ph[:, c, :], x_sb[:, c * F:(c + 1) * F],
                                 wconv[:, li * F:(li + 1) * F], start=True, stop=True)
            h = hp.tile([F, 4, F], FP)
            nc.scalar.mul(h[:], ph[:], 1.0)
            pa = pap.tile([F, NPER], FP, tag="pa")
            for c in range(4):
                nc.tensor.matmul(pa[:], h[:, c, :], m_sb[:, c * NPER:(c + 1) * NPER],
                                 start=(c == 0), stop=False)
            nc.tensor.matmul(pa[:], bconv[0:1, li * F:(li + 1) * F], keep_row[:],
                             start=False, stop=True)
            x_next = xp.tile([F, NPER], FP)
            if residual:
                nc.vector.scalar_tensor_tensor(x_next[:], pa[:], 0.0, x_sb[:],
                                               op0=ALU.max, op1=ALU.add)
            else:
                nc.scalar.activation(x_next[:], pa[:], AF.Relu)
            return x_next

        # ================= per-graph pipeline =================
        for g in range(GPC):
            x_sb = xp.tile([F, NPER], FP)
            nc.sync.dma_start(x_sb[:], d_xT[g * F:(g + 1) * F, :])
            ct_sb = ctp.tile([F, 4 * NPER], FP)
            for c in range(4):
                nc.sync.dma_start(ct_sb[:, c * NPER:(c + 1) * NPER],
                                  d_ct[g * NPER + c * F:g * NPER + (c + 1) * F, :])
            dis_row = sk.tile([1, NPER], FP)
            nc.sync.dma_start(dis_row[:], d_dis0[g:g + 1, :])

            m_sb = build_m(ct_sb, dis_row)
            keep_prev = ones_row

            for p in range(3):
                for li, residual in STAGES[p]:
                    x_sb = conv(x_sb, m_sb, li, keep_prev, residual)

                # ---- pool score ----
                phpw = psp.tile([F, NPER], FP, tag="ps")
                nc.tensor.matmul(phpw[0:1, :], wpool[:, p:p + 1], x_sb[:],
                                 start=True, stop=True)
                hp_row = sk.tile([1, NPER], FP)
                nc.scalar.mul(hp_row[:], phpw[0:1, :], 1.0)
                hp_col = row_to_cols(hp_row)
                psc = psp.tile([F, NPER], FP, tag="ps")
                for c in range(4):
                    nc.tensor.matmul(psc[0:1, :], hp_col[:, c:c + 1],
                                     m_sb[:, c * NPER:(c + 1) * NPER],
                                     start=(c == 0), stop=False, skip_group_check=True)
                nc.tensor.matmul(psc[0:1, :], bpool[0:1, p:p + 1], keep_prev[:],
                                 start=False, stop=True, skip_group_check=True)
                s_eff = sk.tile([1, NPER], FP)
                km1 = sk.tile([1, NPER], FP)
                nc.vector.tensor_scalar(km1[:], keep_prev[0:1, :], -1.0, BIG,
                                        op0=ALU.add, op1=ALU.mult)
                nc.vector.tensor_add(s_eff[:], km1[:], psc[0:1, :])

                # ---- top-k via pairwise rank ----
                sduo = sk.tile([2, NPER], FP)
                nc.vector.tensor_copy(sduo[0:1, :], s_eff[:])
                nc.sync.dma_start(sduo[1:2, :], ones_row[0:1, :])
                negs = sk.tile([1, NPER], FP)
                nc.vector.tensor_scalar_mul(negs[:], s_eff[:], -1.0)
                rduo = sk.tile([2, NPER], FP)
                nc.vector.tensor_copy(rduo[0:1, :], ones_row[0:1, :])
                nc.sync.dma_start(rduo[1:2, :], negs[:])
                prnk = psp.tile([F, NPER], FP, tag="ps")
                for c in range(4):
                    pt = pmp.tile([F, NPER], FP, tag="pm")
                    nc.tensor.matmul(pt[:], sduo[:, c * F:(c + 1) * F], rduo[:],
                                     start=True, stop=True)
                    t_sb = tp.tile([F, NPER], FP)
                    nc.vector.tensor_scalar(t_sb[:], pt[:], 0.0, None, op0=ALU.is_gt)
                    nc.tensor.matmul(prnk[0:1, :], ones_col[:], t_sb[:],
                                     start=(c == 0), stop=(c == 3),
                                     skip_group_check=True)
                keep_row = sk.tile([1, NPER], FP)
                nc.vector.tensor_scalar(keep_row[:], prnk[0:1, :],
                                        float(K_POOL[p]) - 0.5, None, op0=ALU.is_le)

                # ---- x scaling + readout ----
                tanh_s = sk.tile([1, NPER], FP)
                nc.scalar.activation(tanh_s[:], s_eff[:], AF.Tanh)
                txk = sk.tile([1, NPER], FP)
                nc.vector.tensor_mul(txk[:], tanh_s[:], keep_row[:])
                pb = psp.tile([F, NPER], FP, tag="ps")
                nc.tensor.matmul(pb[:], ones_row[0:1, 0:F], txk[:],
                                 start=True, stop=True)
                x_new = xp.tile([F, NPER], FP)
                nc.vector.tensor_mul(x_new[:], x_sb[:], pb[:])
                x_sb = x_new

                negm = sk.tile([1, NPER], FP)
                nc.vector.tensor_scalar(negm[:], keep_row[:], -1.0, BIG,
                                        op0=ALU.add, op1=ALU.mult)
                pnb = psp.tile([F, NPER], FP, tag="ps")
                nc.tensor.matmul(pnb[:], ones_row[0:1, 0:F], negm[:],
                                 start=True, stop=True)
                xm = tp.tile([F, NPER], FP)
                nc.vector.tensor_add(xm[:], x_sb[:], pnb[:])
                mx1 = sk.tile([F, 1], FP)
                nc.vector.tensor_reduce(mx1[:], xm[:], mybir.AxisListType.X, ALU.max)
                sm1 = sk.tile([F, 1], FP)
                nc.vector.tensor_reduce(sm1[:], x_sb[:], mybir.AxisListType.X, ALU.add)
                if p == 0:
                    nc.vector.tensor_copy(R[:, g:g + 1], mx1[:])
                    nc.vector.tensor_scalar_mul(R2[:, g:g + 1], sm1[:],
                                                1.0 / K_POOL[p])
                else:
                    nc.vector.tensor_add(R[:, g:g + 1], R[:, g:g + 1], mx1[:])
                    nc.vector.scalar_tensor_tensor(R2[:, g:g + 1], sm1[:],
                                                   1.0 / K_POOL[p], R2[:, g:g + 1],
                                                   op0=ALU.mult, op1=ALU.add)

                # ---- next-stage M ----
                if p < 2:
                    keep_col = row_to_cols(keep_row)
                    pdeg = psp.tile([F, NPER], FP, tag="ps")
                    for c in range(4):
                        nc.tensor.matmul(pdeg[0:1, :], keep_col[:, c:c + 1],
                                         ct_sb[:, c * NPER:(c + 1) * NPER],
                                         start=(c == 0), stop=(c == 3),
                                         skip_group_check=True)
                    degp = sk.tile([1, NPER], FP)
                    nc.vector.tensor_scalar_add(degp[:], pdeg[0:1, :], 1.0)
                    rec = sk.tile([1, NPER], FP)
                    nc.vector.reciprocal(rec[:], degp[:])
                    sq = sk.tile([1, NPER], FP)
                    nc.scalar.activation(sq[:], rec[:], AF.Sqrt)
                    dis_row = sk.tile([1, NPER], FP)
                    nc.vector.tensor_mul(dis_row[:], sq[:], keep_row[:])
                    m_sb = build_m(ct_sb, dis_row)
                    keep_prev = keep_row

        # ================= MLP head =================
        pl1 = pap.tile([F, NPER], FP, tag="pa")
        nc.tensor.matmul(pl1[0:GPC, 0:F], R[:, 0:GPC], wlin1[:, 0:F],
                         start=True, stop=False)
        nc.tensor.matmul(pl1[0:GPC, 0:F], R2[:, 0:GPC], wlin1[:, F:2 * F],
                         start=False, stop=False)
        nc.tensor.matmul(pl1[0:GPC, 0:F], ones_row[0:1, 0:GPC], bmlp[0:1, 0:F],
                         start=False, stop=True)
        h1 = sk.tile([GPC, F], FP)
        nc.scalar.activation(h1[:], pl1[0:GPC, 0:F], AF.Relu)
        pt1 = pmp.tile([F, NPER], FP, tag="pm")
        nc.tensor.transpose(pt1[0:F, 0:GPC], h1[:], ident[0:GPC, 0:GPC])
        h1t = sk.tile([F, GPC], FP)
        nc.scalar.mul(h1t[:], pt1[0:F, 0:GPC], 1.0)

        pl2 = pap.tile([F, NPER], FP, tag="pa")
        nc.tensor.matmul(pl2[0:GPC, 0:64], h1t[:], wlin2[:], start=True, stop=False)
        nc.tensor.matmul(pl2[0:GPC, 0:64], ones_row[0:1, 0:GPC],
                         bmlp[0:1, F:F + 64], start=False, stop=True)
        h2 = sk.tile([GPC, 64], FP)
        nc.scalar.activation(h2[:], pl2[0:GPC, 0:64], AF.Relu)
        pt2 = pmp.tile([F, NPER], FP, tag="pm")
        nc.tensor.transpose(pt2[0:64, 0:GPC], h2[:], ident[0:GPC, 0:GPC])
        h2t = sk.tile([64, GPC], FP)
        nc.scalar.mul(h2t[:], pt2[0:64, 0:GPC], 1.0)

        pl3 = pap.tile([F, NPER], FP, tag="pa")
        nc.tensor.matmul(pl3[0:GPC, 0:NCLS], h2t[:], wlin3[:], start=True, stop=False)
        nc.tensor.matmul(pl3[0:GPC, 0:NCLS], ones_row[0:1, 0:GPC],
                         bmlp[0:1, 2 * F:2 * F + NCLS], start=False, stop=True)

        mxr = sk.tile([GPC, 1], FP)
        nc.vector.tensor_reduce(mxr[:], pl3[0:GPC, 0:NCLS], mybir.AxisListType.X,
                                ALU.max)
        mxn = sk.tile([GPC, 1], FP)
        nc.vector.tensor_scalar_mul(mxn[:], mxr[:], -1.0)
        e_tmp = sk.tile([GPC, NCLS], FP)
        se = sk.tile([GPC, 1], FP)
        nc.scalar.activation(e_tmp[:], pl3[0:GPC, 0:NCLS], AF.Exp, bias=mxn[:],
                             accum_out=se[:])
        shift = sk.tile([GPC, NCLS], FP)
        nc.vector.tensor_scalar_sub(shift[:], pl3[0:GPC, 0:NCLS], mxr[:])
        lse = sk.tile([GPC, 1], FP)
        nc.scalar.activation(lse[:], se[:], AF.Ln)
        outv = sk.tile([GPC, NCLS], FP)
        nc.vector.tensor_scalar_sub(outv[:], shift[:], lse[:])
        nc.sync.dma_start(d_out[:], outv[:])

    nc.compile()
    return nc


_NC_CACHE = None


def _get_program():
    global _NC_CACHE
    if _NC_CACHE is None:
        _NC_CACHE = _build_program()
    return _NC_CACHE


def _host_prep(x, params, src, dst):
    x = np.ascontiguousarray(np.asarray(x, np.float32))
    src = np.asarray(src).astype(np.int64)
    dst = np.asarray(dst).astype(np.int64)
    P = {k: (np.asarray(v[0], np.float32), np.asarray(v[1], np.float32))
         for k, v in params.items()}

    g = src // NPER
    idx = g * (NPER * NPER) + (src % NPER) * NPER + (dst % NPER)
    counts = np.bincount(idx, minlength=B * NPER * NPER).astype(np.float32)
    Ct = counts.reshape(B, NPER, NPER)  # Ct[g, s, d]
    deg0 = Ct.sum(axis=1) + 1.0
    dis0 = (1.0 / np.sqrt(deg0)).astype(np.float32)

    wconv = np.concatenate([P[f"conv{i}"][0] for i in range(1, 15)], axis=1)
    bconv = np.concatenate(
        [P[f"conv{i}"][1] for i in range(1, 15)]).reshape(1, 14 * F)
    wpool = np.concatenate([P[f"pool{i}"][0] for i in range(1, 4)], axis=1)
    bpool = np.stack([P[f"pool{i}"][1] for i in range(1, 4)], axis=1)  # [1,3]
    W1 = P["lin1"][0]
    wlin1 = np.concatenate([W1[0:F, :], W1[F:2 * F, :]], axis=1)  # [128, 256]
    bmlp = np.zeros((1, 3 * F), np.float32)
    bmlp[0, 0:F] = P["lin1"][1]
    bmlp[0, F:F + 64] = P["lin2"][1]
    bmlp[0, 2 * F:2 * F + NCLS] = P["lin3"][1]
    ident = np.eye(F, dtype=np.float32)
    diagm = np.zeros((F, NPER), np.float32)
    for c in range(4):
        diagm[np.arange(F), c * F + np.arange(F)] = 1.0

    shared = {
        "wconv": np.ascontiguousarray(wconv),
        "bconv": np.ascontiguousarray(bconv),
        "wpool": np.ascontiguousarray(wpool),
        "bpool": np.ascontiguousarray(bpool, dtype=np.float32).reshape(1, 3),
        "wlin1": np.ascontiguousarray(wlin1),
        "wlin2": np.ascontiguousarray(P["lin2"][0]),
        "wlin3": np.ascontiguousarray(P["lin3"][0]),
        "bmlp": bmlp, "ident": ident, "diagm": diagm,
    }
    in_maps = []
    xg = x.reshape(B, NPER, F)
    for c in range(N_CORES):
        gs = slice(c * GPC, (c + 1) * GPC)
        xT = np.ascontiguousarray(
            xg[gs].transpose(0, 2, 1)).reshape(GPC * F, NPER)
        ct = np.ascontiguousarray(Ct[gs]).reshape(GPC * NPER, NPER)
        in_maps.append({"xT": xT, "ct": ct,
                        "dis0": np.ascontiguousarray(dis0[gs]), **shared})
    return in_maps


def kernel(x, params, src, dst):
    nc = _get_program()
    in_maps = _host_prep(x, params, src, dst)
    res = run_bass_kernel_spmd(nc, in_maps, core_ids=list(range(N_CORES)))
    return np.concatenate([res.results[c]["out"] for c in range(N_CORES)], axis=0)
